# revision 13
# baseline (speedup 1.0000x reference)
"""Multi-head causal attention with RoPE on 8 TRN2 NeuronCores.

Sharding: batch (2) x head-groups (4 of 4 heads) -> 8 cores.
Per core, processed per 512-row s-chunk with everything interleaved to keep
the PE array dense: QKV projection for the chunk, RoPE (stream_shuffle +
sign-folded cos/sin, all bf16 on DVE), row-tiled scores S^T = Kr @ Qr^T --
the two heads of a pair run CONCURRENTLY as K=64 matmuls in PE row-groups
0-1 / 2-3 -- with causal block-skip and diagonal-range narrowing, one wide
fused exp over both heads' scores from a 2-bank PSUM tile, a single
128x128 triangle mask applied in-place post-exp, PV matmul with a
ones-column on V accumulating the softmax denominator, DVE reciprocal,
ones-matmul broadcast, then the W_o partial projection for the chunk.
Host sums the 4 per-batch partials.
"""
import os
import sys

sys.path.insert(0, "/opt/trn_rl_repo")

import ml_dtypes
import numpy as np

import concourse.bass as bass
import concourse.mybir as mybir
import concourse.tile as tile
from concourse import bass_utils

F32 = mybir.dt.float32
F32R = mybir.dt.float32r
BF16 = mybir.dt.bfloat16

DT_NAME = os.environ.get("ATTN_DT", "bf16")
DT = {"f32r": F32R, "bf16": BF16}[DT_NAME]
DT_NP = {"f32r": np.float32, "bf16": ml_dtypes.bfloat16}[DT_NAME]

B, S, E, H, Dh = 2, 2048, 1024, 16, 64
HG = 4            # heads per core
HD = HG * Dh      # 256 output channels per core
SCALE = float(1.0 / np.sqrt(np.float32(1024.0)))
ROPE_BASE = 10000.0
NCHUNK = S // 512     # 4 s-chunks of 512
NTB = S // 128        # 16 t-blocks of 128
SHUF16 = list(range(16, 32)) + list(range(0, 16))

Exp = mybir.ActivationFunctionType.Exp
MUL = mybir.AluOpType.mult
ADD = mybir.AluOpType.add


def _build_program():
    nc = bass.Bass("TRN2", target_bir_lowering=False, debug=False)

    xT = nc.dram_tensor("xT", [128, NCHUNK, 8, 512], DT, kind="ExternalInput")
    wq = nc.dram_tensor("wq", [128, 8, HD], DT, kind="ExternalInput")
    wk = nc.dram_tensor("wk", [128, 8, HD], DT, kind="ExternalInput")
    wv = nc.dram_tensor("wv", [128, 8, HD], DT, kind="ExternalInput")
    wo = nc.dram_tensor("wo", [128, 2, E], DT, kind="ExternalInput")
    cosd = nc.dram_tensor("cosd", [128, S], DT, kind="ExternalInput")
    sins = nc.dram_tensor("sins", [128, S], DT, kind="ExternalInput")
    tric = nc.dram_tensor("tric", [128, 128], DT, kind="ExternalInput")
    sel2c = nc.dram_tensor("sel2c", [33, 128], DT, kind="ExternalInput")
    y = nc.dram_tensor("y", [S, E], DT, kind="ExternalOutput")

    with tile.TileContext(nc) as tc:
        with (
            tc.tile_pool(name="persist", bufs=1) as pp,
            tc.tile_pool(name="xchunks", bufs=2) as xp,
            tc.tile_pool(name="ropetmp", bufs=3) as rt,
            tc.tile_pool(name="att_es", bufs=3) as ep,
            tc.tile_pool(name="att_row", bufs=2) as rp,
            tc.tile_pool(name="ystg", bufs=2) as yp,
            tc.tile_pool(name="ps_proj", bufs=1, space="PSUM") as ps1,
            tc.tile_pool(name="ps_sc", bufs=2, space="PSUM") as ps_s,
            tc.tile_pool(name="ps_ot", bufs=1, space="PSUM") as ps_o,
            tc.tile_pool(name="ps_aux", bufs=1, space="PSUM") as ps_a,
        ):
            # ---- persistent tensors ----
            qz = pp.tile([128, 2, S], DT)    # Qr^T per head-pair
            krt = pp.tile([128, 2, S], DT)   # Kr^T
            vau = pp.tile([128, NTB, HG, 65], DT)  # V + ones col per (tb, h)
            ot = pp.tile([128, 2, S], DT)    # O^T normalized

            # critical-path DMAs on the scalar queue: Q/K weights first so
            # the first projection can start as soon as chunk 0 of x lands
            # (x chunks + y output stream on the sync queue).
            wq_sb = pp.tile([128, 8, HD], DT)
            nc.scalar.dma_start(wq_sb[:], wq.ap())
            wk_sb = pp.tile([128, 8, HD], DT)
            nc.scalar.dma_start(wk_sb[:], wk.ap())
            cos_sb = pp.tile([128, S], DT)
            nc.scalar.dma_start(cos_sb[:], cosd.ap())
            sin_sb = pp.tile([128, S], DT)
            nc.scalar.dma_start(sin_sb[:], sins.ap())
            wv_sb = pp.tile([128, 8, HD], DT)
            nc.scalar.dma_start(wv_sb[:], wv.ap())
            # cold-path constants on the gpsimd software DGE
            wo_sb = pp.tile([128, 2, E], DT)
            nc.gpsimd.dma_start(wo_sb[:], wo.ap())
            sel2_sb = pp.tile([33, 128], DT)
            nc.gpsimd.dma_start(sel2_sb[:], sel2c.ap())
            mask_sb = pp.tile([128, 128], DT)
            nc.gpsimd.dma_start(mask_sb[:], tric.ap())

            rows_t = pp.tile([33, 512], DT)
            heat_sb = pp.tile([128, 128], DT)
            nc.vector.memset(heat_sb[:], 0.0)

            def heat(target, n=10):
                # full-array 128x128 matmuls to trip the HAM activity window
                # back to K=8/8. Scratch lands in `target` PSUM, whose next
                # real matmul uses start=True and overwrites it.
                for _ in range(n):
                    nc.tensor.matmul(target[:, 0:128], heat_sb[:],
                                     heat_sb[:], start=True, stop=True)

            ztmp = pp.tile([128, 1], F32)
            nc.vector.memset(ztmp[:], 0.0)
            with nc.allow_low_precision(reason="rounded matmul input"):
                nc.vector.tensor_copy(rows_t[:], ztmp[0:33, :].to_broadcast((33, 512)))

            # warm the PE during the initial DMA streams
            hstart = ps_s.tile([128, 1024], F32, tag="pss", name="heatstart")
            heat(hstart, n=28)

            # ones column of V_aug (free-dim broadcast from a [128,1] slice)
            ones_sb = pp.tile([128, 1], DT)
            with nc.allow_low_precision(reason="rounded matmul input"):
                nc.vector.memset(ones_sb[:], 1.0)
                nc.vector.tensor_copy(
                    vau[:, :, :, 64:65],
                    ones_sb[:, 0:1].to_broadcast((128, NTB, HG, 1)),
                )

            for sc in range(NCHUNK):
                ss = slice(sc * 512, (sc + 1) * 512)
                ntb = 4 * sc + 4

                # ---- projection for this chunk ----
                xc = xp.tile([128, 8, 512], DT, tag="xc")
                nc.sync.dma_start(xc[:], xT.ap()[:, sc])

                # Q and K projections with RoPE (all-bf16 DVE chain -> 2x)
                for w_sb, dst in ((wq_sb, qz), (wk_sb, krt)):
                    for mb in range(2):
                        pq = ps1.tile([128, 512], F32, tag="pq")
                        for e in range(8):
                            nc.tensor.matmul(
                                pq[:], w_sb[:, e, mb * 128:(mb + 1) * 128],
                                xc[:, e, :], start=(e == 0), stop=(e == 7),
                            )
                        a = rt.tile([128, 512], DT, tag="a")
                        with nc.allow_low_precision(reason="rounded matmul input"):
                            nc.vector.tensor_copy(a[:], pq[:])
                        bsh = rt.tile([128, 512], DT, tag="b")
                        nc.vector.stream_shuffle(bsh[:], a[:], SHUF16)
                        t1 = rt.tile([128, 512], DT, tag="t1")
                        t2 = rt.tile([128, 512], DT, tag="t2")
                        with nc.allow_low_precision(reason="rounded matmul input"):
                            nc.vector.tensor_tensor(t1[:], bsh[:], sin_sb[:, ss], MUL)
                            nc.vector.tensor_tensor(t2[:], a[:], cos_sb[:, ss], MUL)
                            nc.vector.tensor_tensor(dst[:, mb, ss], t2[:], t1[:], ADD)

                # V projection (re-uses the pq bank)
                for tbl in range(4):
                    tb = sc * 4 + tbl
                    pvt = ps1.tile([128, 512], F32, tag="pq", name="pv")
                    pv = pvt[:, 0:256]
                    for e in range(8):
                        nc.tensor.matmul(
                            pv, xc[:, e, tbl * 128:(tbl + 1) * 128],
                            wv_sb[:, e, :], start=(e == 0), stop=(e == 7),
                        )
                    with nc.allow_low_precision(reason="rounded matmul input"):
                        nc.scalar.copy(
                            vau[:, tb, :, 0:64],
                            pv.rearrange("p (h d) -> p h d", d=64),
                        )

                # ---- attention for this chunk, one head-pair at a time ----
                # the two heads of a pair run concurrently as K=64 matmuls in
                # PE row-groups 0-1 (partitions 0:64) and 2-3 (64:128), into
                # the two banks of one [128,1024] PSUM tile.
                for hp in range(2):
                    otp = [ps_o.tile([65, 512], F32, tag=f"ot{hi}", name=f"otp{hi}")
                           for hi in range(2)]
                    for tb in range(ntb):
                        m = tb - 4 * sc
                        if sc == 3 and tb % 5 == 1:
                            htile = ps_s.tile([128, 1024], F32, tag="pss", name="heatt")
                            heat(htile, n=5)
                        lo = max(m, 0) * 128  # first valid column (diag blocks)
                        tsl = slice(tb * 128, (tb + 1) * 128)
                        scol = slice(sc * 512 + lo, (sc + 1) * 512)
                        pss = ps_s.tile([128, 1024], F32, tag="pss", name="pss")
                        for hi in range(2):
                            hsl = slice(64 * hi, 64 * hi + 64)
                            nc.tensor.matmul(
                                pss[:, 512 * hi + lo:512 * (hi + 1)],
                                krt[hsl, hp, tsl], qz[hsl, hp, scol],
                                start=True, stop=True,
                            )
                        es = ep.tile([128, 1024], DT, tag="es", name="es")
                        with nc.allow_low_precision(reason="rounded matmul input"):
                            if lo:
                                pr = pss[:].rearrange("p (h s) -> p h s", h=2)
                                er = es[:].rearrange("p (h s) -> p h s", h=2)
                                nc.scalar.activation(er[:, :, lo:512], pr[:, :, lo:512],
                                                     Exp, bias=0.0, scale=SCALE)
                            else:
                                nc.scalar.activation(es[:], pss[:],
                                                     Exp, bias=0.0, scale=SCALE)
                        if m >= 0:  # diagonal block: mask the 128-wide triangles
                            with nc.allow_low_precision(reason="rounded matmul input"):
                                for hi in range(2):
                                    dsl = slice(512 * hi + lo, 512 * hi + lo + 128)
                                    nc.vector.tensor_tensor(
                                        es[:, dsl], es[:, dsl], mask_sb[:], MUL)
                        for hi in range(2):
                            nc.tensor.matmul(
                                otp[hi][:, lo:512], vau[:, tb, 2 * hp + hi, :],
                                es[:, 512 * hi + lo:512 * (hi + 1)],
                                start=(tb == 0), stop=(tb == ntb - 1),
                            )
                    # softmax denominators -> reciprocal on DVE -> one
                    # selector matmul broadcasts both to the full 128-block.
                    with nc.allow_low_precision(reason="rounded matmul input"):
                        for hi in range(2):
                            nc.vector.reciprocal(rows_t[32 * hi:32 * hi + 1, :],
                                                 otp[hi][64:65, :])
                    bc = ps_a.tile([128, 512], F32, tag="aux", name="bc")
                    nc.tensor.matmul(bc[:], sel2_sb[:], rows_t[:], start=True, stop=True)
                    # normalize both heads: O rows are otp[hi][0:64]
                    for hi in range(2):
                        bcs = rp.tile([64, 512], F32, tag="bcs", name=f"bcs{hi}")
                        nc.vector.tensor_copy(bcs[:], bc[hi * 64:(hi + 1) * 64, :])
                        with nc.allow_low_precision(reason="rounded matmul input"):
                            nc.vector.tensor_tensor(ot[hi * 64:(hi + 1) * 64, hp, ss],
                                                    otp[hi][0:64, :], bcs[:], MUL)

                # ---- W_o for this chunk's 4 s-blocks ----
                for sbl in range(4):
                    sb_i = sc * 4 + sbl
                    tsl = slice(sb_i * 128, (sb_i + 1) * 128)
                    ystg = yp.tile([128, E], DT, tag="y")
                    for ec in range(2):
                        py = ps_a.tile([128, 512], F32, tag="aux", name="py")
                        for blk in range(2):
                            nc.tensor.matmul(
                                py[:], ot[:, blk, tsl],
                                wo_sb[:, blk, ec * 512:(ec + 1) * 512],
                                start=(blk == 0), stop=(blk == 1),
                            )
                        with nc.allow_low_precision(reason="rounded matmul input"):
                            if ec == 0:
                                nc.vector.tensor_copy(ystg[:, 0:512], py[:])
                            else:
                                nc.scalar.copy(ystg[:, 512:1024], py[:])
                    nc.sync.dma_start(y.ap()[tsl, :], ystg[:])

    _legalize_waits(nc)
    return nc


def _legalize_waits(nc, max_waits=1):
    """Split >max_waits sync waits onto preceding same-engine NoOps
    (several instruction encodings only have one sync-wait slot)."""
    for fn in nc.m.functions:
        for bb in fn.blocks:
            new_insts = []
            for inst in bb.instructions:
                si = inst.sync_info
                waits = list(si.on_wait) if si is not None and si.on_wait else []
                if len(waits) > max_waits:
                    carry, keep = waits[:-max_waits], waits[-max_waits:]
                    for i, w in enumerate(carry):
                        new_insts.append(mybir.InstNoOp(
                            name=f"{inst.name}_wsplit{i}",
                            engine=inst.engine,
                            bass_nofuse=True,
                            sync_info=mybir.SyncInfo(on_wait=[w], on_update=[]),
                        ))
                    si.on_wait = keep
                new_insts.append(inst)
            bb.instructions[:] = new_insts


def _host_constants():
    # RoPE channel permutation: row r (within a head, 0..63) holds source
    # channel d = 2*i + odd with i = 16*(r//32) + r%16, odd = (r%32)//16.
    r = np.arange(64)
    i_ = 16 * (r // 32) + (r % 16)
    odd = (r % 32) // 16
    dsrc = 2 * i_ + odd  # source channel per permuted row

    inv_freq = ROPE_BASE ** (-(i_.astype(np.float64)) * 2.0 / Dh)
    ang = np.arange(S, dtype=np.float64)[None, :] * inv_freq[:, None]  # [64, S]
    cos64 = np.cos(ang)
    sin64 = np.sin(ang) * np.where(odd == 0, -1.0, 1.0)[:, None]
    cosd = np.tile(cos64, (2, 1)).astype(DT_NP)
    sins = np.tile(sin64, (2, 1)).astype(DT_NP)

    t = np.arange(128)[:, None]
    s = np.arange(128)[None, :]
    tri = (t <= s).astype(DT_NP)

    sel2 = np.zeros((33, 128), DT_NP)
    sel2[0, 0:64] = 1
    sel2[32, 64:128] = 1
    return dsrc, cosd, sins, tri, sel2


_CACHE = {}


def _run(inputs, trace=False):
    if "nc" not in _CACHE:
        _CACHE["nc"] = _build_program()
        _CACHE["consts"] = _host_constants()
    nc = _CACHE["nc"]
    dsrc, cosd, sins, tri, sel2 = _CACHE["consts"]

    x = np.ascontiguousarray(np.asarray(inputs["x"]), dtype=np.float32)
    W_q = np.asarray(inputs["W_q"], dtype=np.float32)
    W_k = np.asarray(inputs["W_k"], dtype=np.float32)
    W_v = np.asarray(inputs["W_v"], dtype=np.float32)
    W_o = np.asarray(inputs["W_o"], dtype=np.float32)

    # x^T per batch in device layout [p, chunk, eo, s] (contiguous DMA lines)
    xT = []
    for b in range(B):
        xb = np.ascontiguousarray(x[b].T).astype(DT_NP)       # [E, S]
        xT.append(np.ascontiguousarray(
            xb.reshape(8, 128, NCHUNK, 512).transpose(1, 2, 0, 3)))

    def wlayout(Wrows):  # [256, E] -> device [128, 8, 256]
        wt = np.ascontiguousarray(Wrows.T).astype(DT_NP)      # [E, 256]
        return np.ascontiguousarray(wt.reshape(8, 128, HD).transpose(1, 0, 2))

    in_maps = []
    for c in range(8):
        b, g = divmod(c, 4)
        heads = np.arange(4 * g, 4 * g + 4)
        rows_qk = (heads[:, None] * 64 + dsrc[None, :]).reshape(-1)   # permuted
        rows_v = (heads[:, None] * 64 + np.arange(64)[None, :]).reshape(-1)
        wot = np.ascontiguousarray(W_o[:, rows_v].T).astype(DT_NP)    # [256, E]
        in_maps.append({
            "xT": xT[b],
            "wq": wlayout(W_q[rows_qk]),
            "wk": wlayout(W_k[rows_qk]),
            "wv": wlayout(W_v[rows_v]),
            "wo": np.ascontiguousarray(wot.reshape(2, 128, E).transpose(1, 0, 2)),
            "cosd": cosd, "sins": sins, "tric": tri, "sel2c": sel2,
        })

    res = bass_utils.run_bass_kernel_spmd(
        nc, in_maps, core_ids=list(range(8)), trace=trace,
    )
    out = np.zeros((B, S, E), np.float32)
    for c in range(8):
        out[c // 4] += res.results[c]["y"].astype(np.float32)
    return out, res


def kernel(**inputs):
    out, _ = _run(inputs, trace=False)
    return out


# revision 28
# speedup vs baseline: 1.1959x; 1.1959x over previous
"""Multi-head causal attention with RoPE on 8 TRN2 NeuronCores.

Sharding: batch (2) x head-groups (4 of 4 heads) -> 8 cores.
Per core, processed per 512-row s-chunk with everything interleaved to keep
the PE array dense: QKV projection for the chunk, RoPE (stream_shuffle +
sign-folded cos/sin, all bf16 on DVE), row-tiled scores S^T = Kr @ Qr^T --
the two heads of a pair run CONCURRENTLY as K=64 matmuls in PE row-groups
0-1 / 2-3 -- with causal block-skip and diagonal-range narrowing, one wide
fused exp over both heads' scores from a 2-bank PSUM tile, a single
128x128 triangle mask applied in-place post-exp, PV matmul with a
ones-column on V accumulating the softmax denominator, DVE reciprocal,
ones-matmul broadcast, then the W_o partial projection for the chunk.
Host sums the 4 per-batch partials.
"""
import os
import sys

sys.path.insert(0, "/opt/trn_rl_repo")

import ml_dtypes
import numpy as np

import concourse.bass as bass
import concourse.mybir as mybir
import concourse.tile as tile
from concourse import bass_utils

F32 = mybir.dt.float32
F32R = mybir.dt.float32r
BF16 = mybir.dt.bfloat16

DT_NAME = os.environ.get("ATTN_DT", "bf16")
DT = {"f32r": F32R, "bf16": BF16}[DT_NAME]
DT_NP = {"f32r": np.float32, "bf16": ml_dtypes.bfloat16}[DT_NAME]

B, S, E, H, Dh = 2, 2048, 1024, 16, 64
HG = 4            # heads per core
HD = HG * Dh      # 256 output channels per core
SCALE = float(1.0 / np.sqrt(np.float32(1024.0)))
ROPE_BASE = 10000.0
NCHUNK = S // 512     # 4 s-chunks of 512
NTB = S // 128        # 16 t-blocks of 128
SHUF16 = list(range(16, 32)) + list(range(0, 16))

Exp = mybir.ActivationFunctionType.Exp
Ln = mybir.ActivationFunctionType.Ln
MUL = mybir.AluOpType.mult
ADD = mybir.AluOpType.add


def _build_program():
    nc = bass.Bass("TRN2", target_bir_lowering=False, debug=False)

    xT = nc.dram_tensor("xT", [128, NCHUNK, 8, 512], DT, kind="ExternalInput")
    wq = nc.dram_tensor("wq", [128, 8, HD], DT, kind="ExternalInput")
    wk = nc.dram_tensor("wk", [128, 8, HD], DT, kind="ExternalInput")
    wv = nc.dram_tensor("wv", [128, 8, HD], DT, kind="ExternalInput")
    wo = nc.dram_tensor("wo", [128, 2, E], DT, kind="ExternalInput")
    cosd = nc.dram_tensor("cosd", [128, S], DT, kind="ExternalInput")
    sins = nc.dram_tensor("sins", [128, S], DT, kind="ExternalInput")
    tric = nc.dram_tensor("tric", [128, 128], DT, kind="ExternalInput")
    sel2c = nc.dram_tensor("sel2c", [33, 128], DT, kind="ExternalInput")
    y = nc.dram_tensor("y", [S, E], DT, kind="ExternalOutput")

    with tile.TileContext(nc) as tc:
        with (
            tc.tile_pool(name="persist", bufs=1) as pp,
            tc.tile_pool(name="xchunks", bufs=2) as xp,
            tc.tile_pool(name="ropetmp", bufs=3) as rt,
            tc.tile_pool(name="att_es", bufs=3) as ep,
            tc.tile_pool(name="att_row", bufs=2) as rp,
            tc.tile_pool(name="ystg", bufs=2) as yp,
            tc.tile_pool(name="ps_proj", bufs=1, space="PSUM") as ps1,
            tc.tile_pool(name="ps_sc", bufs=2, space="PSUM") as ps_s,
            tc.tile_pool(name="ps_ot", bufs=1, space="PSUM") as ps_o,
            tc.tile_pool(name="ps_aux", bufs=1, space="PSUM") as ps_a,
        ):
            # ---- persistent tensors ----
            qz = pp.tile([128, 2, S], DT)    # Qr^T per head-pair
            krt = pp.tile([128, 2, S], DT)   # Kr^T
            vau = pp.tile([128, NTB, HG, 65], DT)  # V + ones col per (tb, h)
            ot = pp.tile([128, 2, S], DT)    # O^T normalized

            # critical-path DMAs on the scalar queue: Q/K weights first so
            # the first projection can start as soon as chunk 0 of x lands
            # (x chunks + y output stream on the sync queue).
            wq_sb = pp.tile([128, 8, HD], DT)
            nc.scalar.dma_start(wq_sb[:], wq.ap())
            wk_sb = pp.tile([128, 8, HD], DT)
            nc.scalar.dma_start(wk_sb[:], wk.ap())
            cos_sb = pp.tile([128, S], DT)
            nc.scalar.dma_start(cos_sb[:], cosd.ap())
            sin_sb = pp.tile([128, S], DT)
            nc.scalar.dma_start(sin_sb[:], sins.ap())
            wv_sb = pp.tile([128, 8, HD], DT)
            nc.scalar.dma_start(wv_sb[:], wv.ap())
            # cold-path constants on the gpsimd software DGE
            wo_sb = pp.tile([128, 2, E], DT)
            nc.gpsimd.dma_start(wo_sb[:], wo.ap())
            sel2_sb = pp.tile([33, 128], DT)
            nc.gpsimd.dma_start(sel2_sb[:], sel2c.ap())
            mask_sb = pp.tile([128, 128], DT)
            nc.gpsimd.dma_start(mask_sb[:], tric.ap())

            # reciprocal rows live at partitions 0 and 32 (DVE/ACT partition
            # shifts must be 32-granular; the denominators sit at p=64)
            rows2 = pp.tile([33, 512], DT)  # bf16 recip rows for bc matmul
            with nc.allow_low_precision(reason="rounded matmul input"):
                nc.vector.memset(rows2[:], 0.0)
            heat_sb = pp.tile([128, 128], DT)
            nc.vector.memset(heat_sb[:], 0.0)

            def heat(target, n=10):
                # full-array 128x128 matmuls to trip the HAM activity window
                # back to K=8/8. Scratch lands in `target` PSUM, whose next
                # real matmul uses start=True and overwrites it.
                for _ in range(n):
                    nc.tensor.matmul(target[:, 0:128], heat_sb[:],
                                     heat_sb[:], start=True, stop=True)

            # warm the PE during the initial DMA streams
            hstart = ps_s.tile([128, 1024], F32, tag="pss", name="heatstart")
            heat(hstart, n=28)

            # ones column of V_aug (free-dim broadcast from a [128,1] slice)
            ones_sb = pp.tile([128, 1], DT)
            with nc.allow_low_precision(reason="rounded matmul input"):
                nc.vector.memset(ones_sb[:], 1.0)
                nc.vector.tensor_copy(
                    vau[:, :, :, 64:65],
                    ones_sb[:, 0:1].to_broadcast((128, NTB, HG, 1)),
                )

            for sc in range(NCHUNK):
                ss = slice(sc * 512, (sc + 1) * 512)
                ntb = 4 * sc + 4

                # ---- projection for this chunk ----
                xc = xp.tile([128, 8, 512], DT, tag="xc")
                nc.sync.dma_start(xc[:], xT.ap()[:, sc])

                # Q and K projections with RoPE (all-bf16 DVE chain -> 2x)
                for w_sb, dst in ((wq_sb, qz), (wk_sb, krt)):
                    for mb in range(2):
                        pq = ps1.tile([128, 512], F32, tag="pq")
                        for e in range(8):
                            nc.tensor.matmul(
                                pq[:], w_sb[:, e, mb * 128:(mb + 1) * 128],
                                xc[:, e, :], start=(e == 0), stop=(e == 7),
                            )
                        a = rt.tile([128, 512], DT, tag="a")
                        with nc.allow_low_precision(reason="rounded matmul input"):
                            nc.vector.tensor_copy(a[:], pq[:])
                        bsh = rt.tile([128, 512], DT, tag="b")
                        nc.vector.stream_shuffle(bsh[:], a[:], SHUF16)
                        t1 = rt.tile([128, 512], DT, tag="t1")
                        t2 = rt.tile([128, 512], DT, tag="t2")
                        with nc.allow_low_precision(reason="rounded matmul input"):
                            nc.vector.tensor_tensor(t1[:], bsh[:], sin_sb[:, ss], MUL)
                            nc.vector.tensor_tensor(t2[:], a[:], cos_sb[:, ss], MUL)
                            nc.vector.tensor_tensor(dst[:, mb, ss], t2[:], t1[:], ADD)

                # V projection (re-uses the pq bank)
                for tbl in range(4):
                    tb = sc * 4 + tbl
                    pvt = ps1.tile([128, 512], F32, tag="pq", name="pv")
                    pv = pvt[:, 0:256]
                    for e in range(8):
                        nc.tensor.matmul(
                            pv, xc[:, e, tbl * 128:(tbl + 1) * 128],
                            wv_sb[:, e, :], start=(e == 0), stop=(e == 7),
                        )
                    with nc.allow_low_precision(reason="rounded matmul input"):
                        nc.scalar.copy(
                            vau[:, tb, :, 0:64],
                            pv.rearrange("p (h d) -> p h d", d=64),
                        )

                # ---- attention for this chunk, one head-pair at a time ----
                # the two heads of a pair run concurrently as K=64 matmuls in
                # PE row-groups 0-1 (partitions 0:64) and 2-3 (64:128), into
                # the two banks of one [128,1024] PSUM tile.
                for hp in range(2):
                    otp = [ps_o.tile([65, 512], F32, tag=f"ot{hi}", name=f"otp{hi}")
                           for hi in range(2)]
                    for tb in range(ntb):
                        m = tb - 4 * sc
                        if sc == 3 and tb % 5 == 1:
                            htile = ps_s.tile([128, 1024], F32, tag="pss", name="heatt")
                            heat(htile, n=5)
                        lo = max(m, 0) * 128  # first valid column (diag blocks)
                        tsl = slice(tb * 128, (tb + 1) * 128)
                        scol = slice(sc * 512 + lo, (sc + 1) * 512)
                        pss = ps_s.tile([128, 1024], F32, tag="pss", name="pss")
                        for hi in range(2):
                            hsl = slice(64 * hi, 64 * hi + 64)
                            nc.tensor.matmul(
                                pss[:, 512 * hi + lo:512 * (hi + 1)],
                                krt[hsl, hp, tsl], qz[hsl, hp, scol],
                                start=True, stop=True,
                            )
                        es = ep.tile([128, 1024], DT, tag="es", name="es")
                        with nc.allow_low_precision(reason="rounded matmul input"):
                            if lo:
                                pr = pss[:].rearrange("p (h s) -> p h s", h=2)
                                er = es[:].rearrange("p (h s) -> p h s", h=2)
                                nc.scalar.activation(er[:, :, lo:512], pr[:, :, lo:512],
                                                     Exp, bias=0.0, scale=SCALE)
                            else:
                                nc.scalar.activation(es[:], pss[:],
                                                     Exp, bias=0.0, scale=SCALE)
                        if m >= 0:  # diagonal block: mask the 128-wide triangles
                            with nc.allow_low_precision(reason="rounded matmul input"):
                                for hi in range(2):
                                    dsl = slice(512 * hi + lo, 512 * hi + lo + 128)
                                    nc.vector.tensor_tensor(
                                        es[:, dsl], es[:, dsl], mask_sb[:], MUL)
                        for hi in range(2):
                            nc.tensor.matmul(
                                otp[hi][:, lo:512], vau[:, tb, 2 * hp + hi, :],
                                es[:, 512 * hi + lo:512 * (hi + 1)],
                                start=(tb == 0), stop=(tb == ntb - 1),
                            )
                    # 1/colsum via ACT exp(-ln); one selector matmul then
                    # broadcasts both rows to the full 128-block.
                    for hi in range(2):
                        lnr = rp.tile([1, 512], F32, tag="lnr", name=f"lnr{hi}")
                        nc.scalar.activation(lnr[:], otp[hi][64:65, :],
                                             Ln, bias=0.0, scale=1.0)
                        with nc.allow_low_precision(reason="rounded matmul input"):
                            nc.scalar.activation(rows2[32 * hi:32 * hi + 1, :],
                                                 lnr[:], Exp, bias=0.0, scale=-1.0)
                    bc = ps_a.tile([128, 512], F32, tag="aux", name="bc")
                    nc.tensor.matmul(bc[:], sel2_sb[:], rows2[:], start=True, stop=True)
                    # normalize both heads: O rows are otp[hi][0:64]
                    for hi in range(2):
                        bcs = rp.tile([64, 512], F32, tag="bcs", name=f"bcs{hi}")
                        nc.vector.tensor_copy(bcs[:], bc[hi * 64:(hi + 1) * 64, :])
                        with nc.allow_low_precision(reason="rounded matmul input"):
                            nc.vector.tensor_tensor(ot[hi * 64:(hi + 1) * 64, hp, ss],
                                                    otp[hi][0:64, :], bcs[:], MUL)

                # ---- W_o for this chunk's 4 s-blocks ----
                for sbl in range(4):
                    sb_i = sc * 4 + sbl
                    tsl = slice(sb_i * 128, (sb_i + 1) * 128)
                    ystg = yp.tile([128, E], DT, tag="y")
                    for ec in range(2):
                        py = ps_a.tile([128, 512], F32, tag="aux", name="py")
                        for blk in range(2):
                            nc.tensor.matmul(
                                py[:], ot[:, blk, tsl],
                                wo_sb[:, blk, ec * 512:(ec + 1) * 512],
                                start=(blk == 0), stop=(blk == 1),
                            )
                        with nc.allow_low_precision(reason="rounded matmul input"):
                            if ec == 0:
                                nc.vector.tensor_copy(ystg[:, 0:512], py[:])
                            else:
                                nc.scalar.copy(ystg[:, 512:1024], py[:])
                    nc.sync.dma_start(y.ap()[tsl, :], ystg[:])

    _legalize_waits(nc)
    return nc


def _legalize_waits(nc, max_waits=1):
    """Split >max_waits sync waits onto preceding same-engine NoOps
    (several instruction encodings only have one sync-wait slot)."""
    for fn in nc.m.functions:
        for bb in fn.blocks:
            new_insts = []
            for inst in bb.instructions:
                si = inst.sync_info
                waits = list(si.on_wait) if si is not None and si.on_wait else []
                if len(waits) > max_waits:
                    carry, keep = waits[:-max_waits], waits[-max_waits:]
                    for i, w in enumerate(carry):
                        new_insts.append(mybir.InstNoOp(
                            name=f"{inst.name}_wsplit{i}",
                            engine=inst.engine,
                            bass_nofuse=True,
                            sync_info=mybir.SyncInfo(on_wait=[w], on_update=[]),
                        ))
                    si.on_wait = keep
                new_insts.append(inst)
            bb.instructions[:] = new_insts


def _host_constants():
    # RoPE channel permutation: row r (within a head, 0..63) holds source
    # channel d = 2*i + odd with i = 16*(r//32) + r%16, odd = (r%32)//16.
    r = np.arange(64)
    i_ = 16 * (r // 32) + (r % 16)
    odd = (r % 32) // 16
    dsrc = 2 * i_ + odd  # source channel per permuted row

    inv_freq = ROPE_BASE ** (-(i_.astype(np.float64)) * 2.0 / Dh)
    ang = np.arange(S, dtype=np.float64)[None, :] * inv_freq[:, None]  # [64, S]
    cos64 = np.cos(ang)
    sin64 = np.sin(ang) * np.where(odd == 0, -1.0, 1.0)[:, None]
    cosd = np.tile(cos64, (2, 1)).astype(DT_NP)
    sins = np.tile(sin64, (2, 1)).astype(DT_NP)

    t = np.arange(128)[:, None]
    s = np.arange(128)[None, :]
    tri = (t <= s).astype(DT_NP)

    sel2 = np.zeros((33, 128), DT_NP)
    sel2[0, 0:64] = 1
    sel2[32, 64:128] = 1
    return dsrc, cosd, sins, tri, sel2


_CACHE = {}


def _run(inputs, trace=False):
    if "nc" not in _CACHE:
        _CACHE["nc"] = _build_program()
        _CACHE["consts"] = _host_constants()
    nc = _CACHE["nc"]
    dsrc, cosd, sins, tri, sel2 = _CACHE["consts"]

    x = np.ascontiguousarray(np.asarray(inputs["x"]), dtype=np.float32)
    W_q = np.asarray(inputs["W_q"], dtype=np.float32)
    W_k = np.asarray(inputs["W_k"], dtype=np.float32)
    W_v = np.asarray(inputs["W_v"], dtype=np.float32)
    W_o = np.asarray(inputs["W_o"], dtype=np.float32)

    # x^T per batch in device layout [p, chunk, eo, s] (contiguous DMA lines)
    xT = []
    for b in range(B):
        xb = np.ascontiguousarray(x[b].T).astype(DT_NP)       # [E, S]
        xT.append(np.ascontiguousarray(
            xb.reshape(8, 128, NCHUNK, 512).transpose(1, 2, 0, 3)))

    def wlayout(Wrows):  # [256, E] -> device [128, 8, 256]
        wt = np.ascontiguousarray(Wrows.T).astype(DT_NP)      # [E, 256]
        return np.ascontiguousarray(wt.reshape(8, 128, HD).transpose(1, 0, 2))

    in_maps = []
    for c in range(8):
        b, g = divmod(c, 4)
        heads = np.arange(4 * g, 4 * g + 4)
        rows_qk = (heads[:, None] * 64 + dsrc[None, :]).reshape(-1)   # permuted
        rows_v = (heads[:, None] * 64 + np.arange(64)[None, :]).reshape(-1)
        wot = np.ascontiguousarray(W_o[:, rows_v].T).astype(DT_NP)    # [256, E]
        in_maps.append({
            "xT": xT[b],
            "wq": wlayout(W_q[rows_qk]),
            "wk": wlayout(W_k[rows_qk]),
            "wv": wlayout(W_v[rows_v]),
            "wo": np.ascontiguousarray(wot.reshape(2, 128, E).transpose(1, 0, 2)),
            "cosd": cosd, "sins": sins, "tric": tri, "sel2c": sel2,
        })

    res = bass_utils.run_bass_kernel_spmd(
        nc, in_maps, core_ids=list(range(8)), trace=trace,
    )
    out = np.zeros((B, S, E), np.float32)
    for c in range(8):
        out[c // 4] += res.results[c]["y"].astype(np.float32)
    return out, res


def kernel(**inputs):
    out, _ = _run(inputs, trace=False)
    return out


# revision 32
# speedup vs baseline: 1.2084x; 1.0104x over previous
"""Multi-head causal attention with RoPE on 8 TRN2 NeuronCores.

Sharding: batch (2) x head-groups (4 of 4 heads) -> 8 cores.
Per core, processed per 512-row s-chunk with everything interleaved to keep
the PE array dense: QKV projection for the chunk, RoPE (stream_shuffle +
sign-folded cos/sin, all bf16 on DVE), row-tiled scores S^T = Kr @ Qr^T --
the two heads of a pair run CONCURRENTLY as K=64 matmuls in PE row-groups
0-1 / 2-3 -- with causal block-skip and diagonal-range narrowing, one wide
fused exp over both heads' scores from a 2-bank PSUM tile, a single
128x128 triangle mask applied in-place post-exp, PV matmul with a
ones-column on V accumulating the softmax denominator, DVE reciprocal,
ones-matmul broadcast, then the W_o partial projection for the chunk.
Host sums the 4 per-batch partials.
"""
import os
import sys

sys.path.insert(0, "/opt/trn_rl_repo")

import ml_dtypes
import numpy as np

import concourse.bass as bass
import concourse.mybir as mybir
import concourse.tile as tile
from concourse import bass_utils

F32 = mybir.dt.float32
F32R = mybir.dt.float32r
BF16 = mybir.dt.bfloat16

DT_NAME = os.environ.get("ATTN_DT", "bf16")
DT = {"f32r": F32R, "bf16": BF16}[DT_NAME]
DT_NP = {"f32r": np.float32, "bf16": ml_dtypes.bfloat16}[DT_NAME]

B, S, E, H, Dh = 2, 2048, 1024, 16, 64
HG = 4            # heads per core
HD = HG * Dh      # 256 output channels per core
SCALE = float(1.0 / np.sqrt(np.float32(1024.0)))
ROPE_BASE = 10000.0
NCHUNK = S // 512     # 4 s-chunks of 512
NTB = S // 128        # 16 t-blocks of 128
SHUF16 = list(range(16, 32)) + list(range(0, 16))

Exp = mybir.ActivationFunctionType.Exp
Ln = mybir.ActivationFunctionType.Ln
MUL = mybir.AluOpType.mult
ADD = mybir.AluOpType.add


def _build_program():
    nc = bass.Bass("TRN2", target_bir_lowering=False, debug=False)

    xT = nc.dram_tensor("xT", [128, NCHUNK, 8, 512], DT, kind="ExternalInput")
    wq = nc.dram_tensor("wq", [128, 8, HD], DT, kind="ExternalInput")
    wk = nc.dram_tensor("wk", [128, 8, HD], DT, kind="ExternalInput")
    wv = nc.dram_tensor("wv", [128, 8, HD], DT, kind="ExternalInput")
    wo = nc.dram_tensor("wo", [128, 2, E], DT, kind="ExternalInput")
    cosd = nc.dram_tensor("cosd", [128, S], DT, kind="ExternalInput")
    sins = nc.dram_tensor("sins", [128, S], DT, kind="ExternalInput")
    tric = nc.dram_tensor("tric", [128, 128], DT, kind="ExternalInput")
    sel2c = nc.dram_tensor("sel2c", [33, 128], DT, kind="ExternalInput")
    y = nc.dram_tensor("y", [S, E], DT, kind="ExternalOutput")

    with tile.TileContext(nc) as tc:
        with (
            tc.tile_pool(name="persist", bufs=1) as pp,
            tc.tile_pool(name="xchunks", bufs=2) as xp,
            tc.tile_pool(name="ropetmp", bufs=3) as rt,
            tc.tile_pool(name="att_es", bufs=3) as ep,
            tc.tile_pool(name="att_row", bufs=2) as rp,
            tc.tile_pool(name="ystg", bufs=2) as yp,
            tc.tile_pool(name="ps_proj", bufs=1, space="PSUM") as ps1,
            tc.tile_pool(name="ps_sc", bufs=2, space="PSUM") as ps_s,
            tc.tile_pool(name="ps_ot", bufs=1, space="PSUM") as ps_o,
            tc.tile_pool(name="ps_aux", bufs=1, space="PSUM") as ps_a,
        ):
            # ---- persistent tensors ----
            qz = pp.tile([128, 2, S], DT)    # Qr^T per head-pair
            krt = pp.tile([128, 2, S], DT)   # Kr^T
            vau = pp.tile([128, NTB, HG, 65], DT)  # V + ones col per (tb, h)
            ot = pp.tile([128, 2, S], DT)    # O^T normalized

            # critical-path DMAs on the scalar queue: Q/K weights first so
            # the first projection can start as soon as chunk 0 of x lands
            # (x chunks + y output stream on the sync queue).
            wq_sb = pp.tile([128, 8, HD], DT)
            nc.scalar.dma_start(wq_sb[:], wq.ap())
            wk_sb = pp.tile([128, 8, HD], DT)
            nc.scalar.dma_start(wk_sb[:], wk.ap())
            cos_sb = pp.tile([128, S], DT)
            nc.scalar.dma_start(cos_sb[:], cosd.ap())
            sin_sb = pp.tile([128, S], DT)
            nc.scalar.dma_start(sin_sb[:], sins.ap())
            wv_sb = pp.tile([128, 8, HD], DT)
            nc.scalar.dma_start(wv_sb[:], wv.ap())
            # cold-path constants on the gpsimd software DGE
            wo_sb = pp.tile([128, 2, E], DT)
            nc.gpsimd.dma_start(wo_sb[:], wo.ap())
            sel2_sb = pp.tile([33, 128], DT)
            nc.gpsimd.dma_start(sel2_sb[:], sel2c.ap())
            mask_sb = pp.tile([128, 128], DT)
            nc.gpsimd.dma_start(mask_sb[:], tric.ap())

            # reciprocal rows live at partitions 0 and 32 (DVE/ACT partition
            # shifts must be 32-granular; the denominators sit at p=64)
            rows2 = pp.tile([33, 512], DT)  # bf16 recip rows for bc matmul
            with nc.allow_low_precision(reason="rounded matmul input"):
                nc.vector.memset(rows2[:], 0.0)
            heat_sb = pp.tile([128, 128], DT)
            nc.vector.memset(heat_sb[:], 0.0)

            def heat(target, n=10):
                # full-array 128x128 matmuls to trip the HAM activity window
                # back to K=8/8. Scratch lands in `target` PSUM, whose next
                # real matmul uses start=True and overwrites it.
                for _ in range(n):
                    nc.tensor.matmul(target[:, 0:128], heat_sb[:],
                                     heat_sb[:], start=True, stop=True)

            # warm the PE during the initial DMA streams; sized to bridge
            # until the first x chunk + W_q land (~13us)
            hstart = ps_s.tile([128, 1024], F32, tag="pss", name="heatstart")
            heat(hstart, n=76)

            # ones column of V_aug (free-dim broadcast from a [128,1] slice)
            ones_sb = pp.tile([128, 1], DT)
            with nc.allow_low_precision(reason="rounded matmul input"):
                nc.vector.memset(ones_sb[:], 1.0)
                nc.vector.tensor_copy(
                    vau[:, :, :, 64:65],
                    ones_sb[:, 0:1].to_broadcast((128, NTB, HG, 1)),
                )

            def emit_wo(sb_i, act_side):
                # W_o partial projection for one 128-row s-block
                tsl = slice(sb_i * 128, (sb_i + 1) * 128)
                ystg = yp.tile([128, E], DT, tag="y")
                for ec in range(2):
                    pool, tg = (ps_a, "aux") if ec == 0 else (ps1, "pq")
                    py = pool.tile([128, 512], F32, tag=tg, name="py")
                    for blk in range(2):
                        nc.tensor.matmul(
                            py[:], ot[:, blk, tsl],
                            wo_sb[:, blk, ec * 512:(ec + 1) * 512],
                            start=(blk == 0), stop=(blk == 1),
                        )
                    with nc.allow_low_precision(reason="rounded matmul input"):
                        if ec == 1 and act_side:
                            nc.scalar.copy(ystg[:, 512:1024], py[:])
                        else:
                            nc.vector.tensor_copy(
                                ystg[:, ec * 512:(ec + 1) * 512], py[:])
                nc.sync.dma_start(y.ap()[tsl, :], ystg[:])

            wo_pending = []  # W_o s-blocks deferred into the next chunk

            for sc in range(NCHUNK):
                ss = slice(sc * 512, (sc + 1) * 512)
                ntb = 4 * sc + 4

                # ---- projection for this chunk (x halves on 2 DMA queues) ----
                xc = xp.tile([128, 8, 512], DT, tag="xc")
                nc.sync.dma_start(xc[:, 0:4], xT.ap()[:, sc, 0:4])
                nc.scalar.dma_start(xc[:, 4:8], xT.ap()[:, sc, 4:8])

                # Q and K projections with RoPE (all-bf16 DVE chain -> 2x)
                for w_sb, dst in ((wq_sb, qz), (wk_sb, krt)):
                    for mb in range(2):
                        pq = ps1.tile([128, 512], F32, tag="pq")
                        for e in range(8):
                            nc.tensor.matmul(
                                pq[:], w_sb[:, e, mb * 128:(mb + 1) * 128],
                                xc[:, e, :], start=(e == 0), stop=(e == 7),
                            )
                        a = rt.tile([128, 512], DT, tag="a")
                        with nc.allow_low_precision(reason="rounded matmul input"):
                            nc.vector.tensor_copy(a[:], pq[:])
                        bsh = rt.tile([128, 512], DT, tag="b")
                        nc.vector.stream_shuffle(bsh[:], a[:], SHUF16)
                        t1 = rt.tile([128, 512], DT, tag="t1")
                        t2 = rt.tile([128, 512], DT, tag="t2")
                        with nc.allow_low_precision(reason="rounded matmul input"):
                            nc.vector.tensor_tensor(t1[:], bsh[:], sin_sb[:, ss], MUL)
                            nc.vector.tensor_tensor(t2[:], a[:], cos_sb[:, ss], MUL)
                            nc.vector.tensor_tensor(dst[:, mb, ss], t2[:], t1[:], ADD)

                # V projection (re-uses the pq bank)
                for tbl in range(4):
                    tb = sc * 4 + tbl
                    pvt = ps1.tile([128, 512], F32, tag="pq", name="pv")
                    pv = pvt[:, 0:256]
                    for e in range(8):
                        nc.tensor.matmul(
                            pv, xc[:, e, tbl * 128:(tbl + 1) * 128],
                            wv_sb[:, e, :], start=(e == 0), stop=(e == 7),
                        )
                    with nc.allow_low_precision(reason="rounded matmul input"):
                        nc.scalar.copy(
                            vau[:, tb, :, 0:64],
                            pv.rearrange("p (h d) -> p h d", d=64),
                        )

                # ---- attention for this chunk, one head-pair at a time ----
                # the two heads of a pair run concurrently as K=64 matmuls in
                # PE row-groups 0-1 (partitions 0:64) and 2-3 (64:128), into
                # the two banks of one [128,1024] PSUM tile.
                wo_at = {ntb // 3, (2 * ntb) // 3}  # sprinkle points per hp
                for hp in range(2):
                    otp = [ps_o.tile([65, 512], F32, tag=f"ot{hi}", name=f"otp{hi}")
                           for hi in range(2)]
                    for tb in range(ntb):
                        m = tb - 4 * sc
                        if tb in wo_at and wo_pending:
                            emit_wo(wo_pending.pop(0), act_side=False)
                        lo = max(m, 0) * 128  # first valid column (diag blocks)
                        tsl = slice(tb * 128, (tb + 1) * 128)
                        scol = slice(sc * 512 + lo, (sc + 1) * 512)
                        pss = ps_s.tile([128, 1024], F32, tag="pss", name="pss")
                        for hi in range(2):
                            hsl = slice(64 * hi, 64 * hi + 64)
                            nc.tensor.matmul(
                                pss[:, 512 * hi + lo:512 * (hi + 1)],
                                krt[hsl, hp, tsl], qz[hsl, hp, scol],
                                start=True, stop=True,
                            )
                        es = ep.tile([128, 1024], DT, tag="es", name="es")
                        with nc.allow_low_precision(reason="rounded matmul input"):
                            if lo:
                                pr = pss[:].rearrange("p (h s) -> p h s", h=2)
                                er = es[:].rearrange("p (h s) -> p h s", h=2)
                                nc.scalar.activation(er[:, :, lo:512], pr[:, :, lo:512],
                                                     Exp, bias=0.0, scale=SCALE)
                            else:
                                nc.scalar.activation(es[:], pss[:],
                                                     Exp, bias=0.0, scale=SCALE)
                        if m >= 0:  # diagonal block: mask the 128-wide triangles
                            with nc.allow_low_precision(reason="rounded matmul input"):
                                for hi in range(2):
                                    dsl = slice(512 * hi + lo, 512 * hi + lo + 128)
                                    nc.vector.tensor_tensor(
                                        es[:, dsl], es[:, dsl], mask_sb[:], MUL)
                        for hi in range(2):
                            nc.tensor.matmul(
                                otp[hi][:, lo:512], vau[:, tb, 2 * hp + hi, :],
                                es[:, 512 * hi + lo:512 * (hi + 1)],
                                start=(tb == 0), stop=(tb == ntb - 1),
                            )
                    # 1/colsum via ACT exp(-ln); one selector matmul then
                    # broadcasts both rows to the full 128-block.
                    for hi in range(2):
                        lnr = rp.tile([1, 512], F32, tag="lnr", name=f"lnr{hi}")
                        nc.scalar.activation(lnr[:], otp[hi][64:65, :],
                                             Ln, bias=0.0, scale=1.0)
                        with nc.allow_low_precision(reason="rounded matmul input"):
                            nc.scalar.activation(rows2[32 * hi:32 * hi + 1, :],
                                                 lnr[:], Exp, bias=0.0, scale=-1.0)
                    bc = ps_a.tile([128, 512], F32, tag="aux", name="bc")
                    nc.tensor.matmul(bc[:], sel2_sb[:], rows2[:], start=True, stop=True)
                    # normalize both heads: O rows are otp[hi][0:64]
                    for hi in range(2):
                        bcs = rp.tile([64, 512], F32, tag="bcs", name=f"bcs{hi}")
                        nc.vector.tensor_copy(bcs[:], bc[hi * 64:(hi + 1) * 64, :])
                        with nc.allow_low_precision(reason="rounded matmul input"):
                            nc.vector.tensor_tensor(ot[hi * 64:(hi + 1) * 64, hp, ss],
                                                    otp[hi][0:64, :], bcs[:], MUL)

                # ---- W_o for this chunk: any leftovers from the previous
                # chunk, then defer this chunk's 4 s-blocks into the next
                # chunk's attention (keeps the PE fed through ACT stalls) ----
                for sb_i in wo_pending:
                    emit_wo(sb_i, act_side=True)
                wo_pending = [sc * 4 + sbl for sbl in range(4)]

            for sb_i in wo_pending:  # last chunk's W_o at the tail
                emit_wo(sb_i, act_side=True)

    _legalize_waits(nc)
    return nc


def _legalize_waits(nc, max_waits=1):
    """Split >max_waits sync waits onto preceding same-engine NoOps
    (several instruction encodings only have one sync-wait slot)."""
    for fn in nc.m.functions:
        for bb in fn.blocks:
            new_insts = []
            for inst in bb.instructions:
                si = inst.sync_info
                waits = list(si.on_wait) if si is not None and si.on_wait else []
                if len(waits) > max_waits:
                    carry, keep = waits[:-max_waits], waits[-max_waits:]
                    for i, w in enumerate(carry):
                        new_insts.append(mybir.InstNoOp(
                            name=f"{inst.name}_wsplit{i}",
                            engine=inst.engine,
                            bass_nofuse=True,
                            sync_info=mybir.SyncInfo(on_wait=[w], on_update=[]),
                        ))
                    si.on_wait = keep
                new_insts.append(inst)
            bb.instructions[:] = new_insts


def _host_constants():
    # RoPE channel permutation: row r (within a head, 0..63) holds source
    # channel d = 2*i + odd with i = 16*(r//32) + r%16, odd = (r%32)//16.
    r = np.arange(64)
    i_ = 16 * (r // 32) + (r % 16)
    odd = (r % 32) // 16
    dsrc = 2 * i_ + odd  # source channel per permuted row

    inv_freq = ROPE_BASE ** (-(i_.astype(np.float64)) * 2.0 / Dh)
    ang = np.arange(S, dtype=np.float64)[None, :] * inv_freq[:, None]  # [64, S]
    cos64 = np.cos(ang)
    sin64 = np.sin(ang) * np.where(odd == 0, -1.0, 1.0)[:, None]
    cosd = np.tile(cos64, (2, 1)).astype(DT_NP)
    sins = np.tile(sin64, (2, 1)).astype(DT_NP)

    t = np.arange(128)[:, None]
    s = np.arange(128)[None, :]
    tri = (t <= s).astype(DT_NP)

    sel2 = np.zeros((33, 128), DT_NP)
    sel2[0, 0:64] = 1
    sel2[32, 64:128] = 1
    return dsrc, cosd, sins, tri, sel2


_CACHE = {}


def _run(inputs, trace=False):
    if "nc" not in _CACHE:
        _CACHE["nc"] = _build_program()
        _CACHE["consts"] = _host_constants()
    nc = _CACHE["nc"]
    dsrc, cosd, sins, tri, sel2 = _CACHE["consts"]

    x = np.ascontiguousarray(np.asarray(inputs["x"]), dtype=np.float32)
    W_q = np.asarray(inputs["W_q"], dtype=np.float32)
    W_k = np.asarray(inputs["W_k"], dtype=np.float32)
    W_v = np.asarray(inputs["W_v"], dtype=np.float32)
    W_o = np.asarray(inputs["W_o"], dtype=np.float32)

    # x^T per batch in device layout [p, chunk, eo, s] (contiguous DMA lines)
    xT = []
    for b in range(B):
        xb = np.ascontiguousarray(x[b].T).astype(DT_NP)       # [E, S]
        xT.append(np.ascontiguousarray(
            xb.reshape(8, 128, NCHUNK, 512).transpose(1, 2, 0, 3)))

    def wlayout(Wrows):  # [256, E] -> device [128, 8, 256]
        wt = np.ascontiguousarray(Wrows.T).astype(DT_NP)      # [E, 256]
        return np.ascontiguousarray(wt.reshape(8, 128, HD).transpose(1, 0, 2))

    in_maps = []
    for c in range(8):
        b, g = divmod(c, 4)
        heads = np.arange(4 * g, 4 * g + 4)
        rows_qk = (heads[:, None] * 64 + dsrc[None, :]).reshape(-1)   # permuted
        rows_v = (heads[:, None] * 64 + np.arange(64)[None, :]).reshape(-1)
        wot = np.ascontiguousarray(W_o[:, rows_v].T).astype(DT_NP)    # [256, E]
        in_maps.append({
            "xT": xT[b],
            "wq": wlayout(W_q[rows_qk]),
            "wk": wlayout(W_k[rows_qk]),
            "wv": wlayout(W_v[rows_v]),
            "wo": np.ascontiguousarray(wot.reshape(2, 128, E).transpose(1, 0, 2)),
            "cosd": cosd, "sins": sins, "tric": tri, "sel2c": sel2,
        })

    res = bass_utils.run_bass_kernel_spmd(
        nc, in_maps, core_ids=list(range(8)), trace=trace,
    )
    out = np.zeros((B, S, E), np.float32)
    for c in range(8):
        out[c // 4] += res.results[c]["y"].astype(np.float32)
    return out, res


def kernel(**inputs):
    out, _ = _run(inputs, trace=False)
    return out


# revision 36
# speedup vs baseline: 1.2790x; 1.0584x over previous
"""Multi-head causal attention with RoPE on 8 TRN2 NeuronCores.

Sharding: batch (2) x head-groups (4 of 4 heads) -> 8 cores.
Per core, processed per 512-row s-chunk with everything interleaved to keep
the PE array dense: QKV projection for the chunk, RoPE (stream_shuffle +
sign-folded cos/sin, all bf16 on DVE), row-tiled scores S^T = Kr @ Qr^T --
the two heads of a pair run CONCURRENTLY as K=64 matmuls in PE row-groups
0-1 / 2-3 -- with causal block-skip and diagonal-range narrowing, one wide
fused exp over both heads' scores from a 2-bank PSUM tile, a single
128x128 triangle mask applied in-place post-exp, PV matmul with a
ones-column on V accumulating the softmax denominator, DVE reciprocal,
ones-matmul broadcast, then the W_o partial projection for the chunk.
Host sums the 4 per-batch partials.
"""
import os
import sys

sys.path.insert(0, "/opt/trn_rl_repo")

import ml_dtypes
import numpy as np

import concourse.bass as bass
import concourse.mybir as mybir
import concourse.tile as tile
from concourse import bass_utils

F32 = mybir.dt.float32
F32R = mybir.dt.float32r
BF16 = mybir.dt.bfloat16

DT_NAME = os.environ.get("ATTN_DT", "bf16")
DT = {"f32r": F32R, "bf16": BF16}[DT_NAME]
DT_NP = {"f32r": np.float32, "bf16": ml_dtypes.bfloat16}[DT_NAME]

B, S, E, H, Dh = 2, 2048, 1024, 16, 64
HG = 4            # heads per core
HD = HG * Dh      # 256 output channels per core
SCALE = float(1.0 / np.sqrt(np.float32(1024.0)))
ROPE_BASE = 10000.0
NCHUNK = S // 512     # 4 s-chunks of 512
NTB = S // 128        # 16 t-blocks of 128
SHUF16 = list(range(16, 32)) + list(range(0, 16))

Exp = mybir.ActivationFunctionType.Exp
Ln = mybir.ActivationFunctionType.Ln
MUL = mybir.AluOpType.mult
ADD = mybir.AluOpType.add


def _build_program():
    nc = bass.Bass("TRN2", target_bir_lowering=False, debug=False)

    xT = nc.dram_tensor("xT", [128, NCHUNK, 8, 512], DT, kind="ExternalInput")
    wq = nc.dram_tensor("wq", [128, 8, HD], DT, kind="ExternalInput")
    wk = nc.dram_tensor("wk", [128, 8, HD], DT, kind="ExternalInput")
    wv = nc.dram_tensor("wv", [128, 8, HD], DT, kind="ExternalInput")
    wo = nc.dram_tensor("wo", [128, 2, E], DT, kind="ExternalInput")
    cosd = nc.dram_tensor("cosd", [128, S], DT, kind="ExternalInput")
    sins = nc.dram_tensor("sins", [128, S], DT, kind="ExternalInput")
    tric = nc.dram_tensor("tric", [128, 128], DT, kind="ExternalInput")
    sel2c = nc.dram_tensor("sel2c", [33, 128], DT, kind="ExternalInput")
    y = nc.dram_tensor("y", [S, E], DT, kind="ExternalOutput")

    with tile.TileContext(nc) as tc:
        with (
            tc.tile_pool(name="persist", bufs=1) as pp,
            tc.tile_pool(name="xchunks", bufs=2) as xp,
            tc.tile_pool(name="ropetmp", bufs=3) as rt,
            tc.tile_pool(name="att_es", bufs=3) as ep,
            tc.tile_pool(name="att_row", bufs=2) as rp,
            tc.tile_pool(name="ystg", bufs=2) as yp,
            tc.tile_pool(name="ps_proj", bufs=1, space="PSUM") as ps1,
            tc.tile_pool(name="ps_sc", bufs=2, space="PSUM") as ps_s,
            tc.tile_pool(name="ps_ot", bufs=1, space="PSUM") as ps_o,
            tc.tile_pool(name="ps_aux", bufs=1, space="PSUM") as ps_a,
        ):
            # ---- persistent tensors ----
            qz = pp.tile([128, 2, S], DT)    # Qr^T per head-pair
            krt = pp.tile([128, 2, S], DT)   # Kr^T
            vau = pp.tile([128, NTB, HG, 65], DT)  # V + ones col per (tb, h)
            ot = pp.tile([128, 2, S], DT)    # O^T normalized

            # critical-path DMAs on the scalar queue: Q/K weights first so
            # the first projection can start as soon as chunk 0 of x lands
            # (x chunks + y output stream on the sync queue).
            wq_sb = pp.tile([128, 8, HD], DT)
            nc.scalar.dma_start(wq_sb[:], wq.ap())
            wk_sb = pp.tile([128, 8, HD], DT)
            nc.scalar.dma_start(wk_sb[:], wk.ap())
            cos_sb = pp.tile([128, S], DT)
            nc.scalar.dma_start(cos_sb[:], cosd.ap())
            sin_sb = pp.tile([128, S], DT)
            nc.scalar.dma_start(sin_sb[:], sins.ap())
            # V/O weights + cold-path constants on the gpsimd software DGE
            wv_sb = pp.tile([128, 8, HD], DT)
            nc.gpsimd.dma_start(wv_sb[:], wv.ap())
            wo_sb = pp.tile([128, 2, E], DT)
            nc.gpsimd.dma_start(wo_sb[:], wo.ap())
            sel2_sb = pp.tile([33, 128], DT)
            nc.gpsimd.dma_start(sel2_sb[:], sel2c.ap())
            mask_sb = pp.tile([128, 128], DT)
            nc.gpsimd.dma_start(mask_sb[:], tric.ap())

            # reciprocal rows live at partitions 0 and 32 (DVE/ACT partition
            # shifts must be 32-granular; the denominators sit at p=64)
            rows2 = pp.tile([33, 512], DT)  # bf16 recip rows for bc matmul
            with nc.allow_low_precision(reason="rounded matmul input"):
                nc.vector.memset(rows2[:], 0.0)
            heat_sb = pp.tile([128, 128], DT)
            nc.vector.memset(heat_sb[:], 0.0)

            def heat(target, n=10):
                # full-array 128x128 matmuls to trip the HAM activity window
                # back to K=8/8. Scratch lands in `target` PSUM, whose next
                # real matmul uses start=True and overwrites it.
                for _ in range(n):
                    nc.tensor.matmul(target[:, 0:128], heat_sb[:],
                                     heat_sb[:], start=True, stop=True)

            # warm the PE during the initial DMA streams; sized to bridge
            # until the first x chunk + W_q land (~13us)
            hstart = ps_s.tile([128, 1024], F32, tag="pss", name="heatstart")
            heat(hstart, n=76)

            # ones column of V_aug (free-dim broadcast from a [128,1] slice)
            ones_sb = pp.tile([128, 1], DT)
            with nc.allow_low_precision(reason="rounded matmul input"):
                nc.vector.memset(ones_sb[:], 1.0)
                nc.vector.tensor_copy(
                    vau[:, :, :, 64:65],
                    ones_sb[:, 0:1].to_broadcast((128, NTB, HG, 1)),
                )

            def emit_wo(sb_i, act_side):
                # W_o partial projection for one 128-row s-block
                tsl = slice(sb_i * 128, (sb_i + 1) * 128)
                ystg = yp.tile([128, E], DT, tag="y")
                for ec in range(2):
                    pool, tg = (ps_a, "aux") if ec == 0 else (ps1, "pq")
                    py = pool.tile([128, 512], F32, tag=tg, name="py")
                    for blk in range(2):
                        nc.tensor.matmul(
                            py[:], ot[:, blk, tsl],
                            wo_sb[:, blk, ec * 512:(ec + 1) * 512],
                            start=(blk == 0), stop=(blk == 1),
                        )
                    with nc.allow_low_precision(reason="rounded matmul input"):
                        if ec == 1 and act_side:
                            nc.scalar.copy(ystg[:, 512:1024], py[:])
                        else:
                            nc.vector.tensor_copy(
                                ystg[:, ec * 512:(ec + 1) * 512], py[:])
                nc.sync.dma_start(y.ap()[tsl, :], ystg[:])

            wo_pending = []  # W_o s-blocks deferred into the next chunk

            for sc in range(NCHUNK):
                ss = slice(sc * 512, (sc + 1) * 512)
                ntb = 4 * sc + 4

                # ---- projection for this chunk (x alone on the sync queue) ----
                xc = xp.tile([128, 8, 512], DT, tag="xc")
                nc.sync.dma_start(xc[:], xT.ap()[:, sc])

                # Q and K projections with RoPE (all-bf16 DVE chain -> 2x)
                for w_sb, dst in ((wq_sb, qz), (wk_sb, krt)):
                    for mb in range(2):
                        pq = ps1.tile([128, 512], F32, tag="pq")
                        for e in range(8):
                            nc.tensor.matmul(
                                pq[:], w_sb[:, e, mb * 128:(mb + 1) * 128],
                                xc[:, e, :], start=(e == 0), stop=(e == 7),
                            )
                        a = rt.tile([128, 512], DT, tag="a")
                        with nc.allow_low_precision(reason="rounded matmul input"):
                            nc.vector.tensor_copy(a[:], pq[:])
                        bsh = rt.tile([128, 512], DT, tag="b")
                        nc.vector.stream_shuffle(bsh[:], a[:], SHUF16)
                        t1 = rt.tile([128, 512], DT, tag="t1")
                        t2 = rt.tile([128, 512], DT, tag="t2")
                        with nc.allow_low_precision(reason="rounded matmul input"):
                            nc.vector.tensor_tensor(t1[:], bsh[:], sin_sb[:, ss], MUL)
                            nc.vector.tensor_tensor(t2[:], a[:], cos_sb[:, ss], MUL)
                            nc.vector.tensor_tensor(dst[:, mb, ss], t2[:], t1[:], ADD)

                # V projection (re-uses the pq bank)
                for tbl in range(4):
                    tb = sc * 4 + tbl
                    pvt = ps1.tile([128, 512], F32, tag="pq", name="pv")
                    pv = pvt[:, 0:256]
                    for e in range(8):
                        nc.tensor.matmul(
                            pv, xc[:, e, tbl * 128:(tbl + 1) * 128],
                            wv_sb[:, e, :], start=(e == 0), stop=(e == 7),
                        )
                    with nc.allow_low_precision(reason="rounded matmul input"):
                        nc.scalar.copy(
                            vau[:, tb, :, 0:64],
                            pv.rearrange("p (h d) -> p h d", d=64),
                        )

                # ---- attention for this chunk, one head-pair at a time ----
                # the two heads of a pair run concurrently as K=64 matmuls in
                # PE row-groups 0-1 (partitions 0:64) and 2-3 (64:128), into
                # the two banks of one [128,1024] PSUM tile.
                wo_at = {ntb // 2}  # mid-hp sprinkle point
                for hp in range(2):
                    otp = [ps_o.tile([65, 512], F32, tag=f"ot{hi}", name=f"otp{hi}")
                           for hi in range(2)]
                    for tb in range(ntb):
                        m = tb - 4 * sc
                        if tb in wo_at and wo_pending:
                            emit_wo(wo_pending.pop(0), act_side=False)
                        lo = max(m, 0) * 128  # first valid column (diag blocks)
                        tsl = slice(tb * 128, (tb + 1) * 128)
                        scol = slice(sc * 512 + lo, (sc + 1) * 512)
                        pss = ps_s.tile([128, 1024], F32, tag="pss", name="pss")
                        for hi in range(2):
                            hsl = slice(64 * hi, 64 * hi + 64)
                            nc.tensor.matmul(
                                pss[:, 512 * hi + lo:512 * (hi + 1)],
                                krt[hsl, hp, tsl], qz[hsl, hp, scol],
                                start=True, stop=True,
                            )
                        es = ep.tile([128, 1024], DT, tag="es", name="es")
                        with nc.allow_low_precision(reason="rounded matmul input"):
                            if lo:
                                pr = pss[:].rearrange("p (h s) -> p h s", h=2)
                                er = es[:].rearrange("p (h s) -> p h s", h=2)
                                nc.scalar.activation(er[:, :, lo:512], pr[:, :, lo:512],
                                                     Exp, bias=0.0, scale=SCALE)
                            else:
                                nc.scalar.activation(es[:], pss[:],
                                                     Exp, bias=0.0, scale=SCALE)
                        if m >= 0:  # diagonal block: mask the 128-wide triangles
                            with nc.allow_low_precision(reason="rounded matmul input"):
                                for hi in range(2):
                                    dsl = slice(512 * hi + lo, 512 * hi + lo + 128)
                                    nc.vector.tensor_tensor(
                                        es[:, dsl], es[:, dsl], mask_sb[:], MUL)
                        for hi in range(2):
                            nc.tensor.matmul(
                                otp[hi][:, lo:512], vau[:, tb, 2 * hp + hi, :],
                                es[:, 512 * hi + lo:512 * (hi + 1)],
                                start=(tb == 0), stop=(tb == ntb - 1),
                            )
                    # W_o filler covers the ln/exp latency before bc
                    if wo_pending:
                        emit_wo(wo_pending.pop(0), act_side=False)
                    # 1/colsum via ACT exp(-ln); one selector matmul then
                    # broadcasts both rows to the full 128-block.
                    for hi in range(2):
                        lnr = rp.tile([1, 512], F32, tag="lnr", name=f"lnr{hi}")
                        nc.scalar.activation(lnr[:], otp[hi][64:65, :],
                                             Ln, bias=0.0, scale=1.0)
                        with nc.allow_low_precision(reason="rounded matmul input"):
                            nc.scalar.activation(rows2[32 * hi:32 * hi + 1, :],
                                                 lnr[:], Exp, bias=0.0, scale=-1.0)
                    bc = ps_a.tile([128, 512], F32, tag="aux", name="bc")
                    nc.tensor.matmul(bc[:], sel2_sb[:], rows2[:], start=True, stop=True)
                    # normalize both heads: O rows are otp[hi][0:64]
                    for hi in range(2):
                        bcs = rp.tile([64, 512], F32, tag="bcs", name=f"bcs{hi}")
                        nc.vector.tensor_copy(bcs[:], bc[hi * 64:(hi + 1) * 64, :])
                        with nc.allow_low_precision(reason="rounded matmul input"):
                            nc.vector.tensor_tensor(ot[hi * 64:(hi + 1) * 64, hp, ss],
                                                    otp[hi][0:64, :], bcs[:], MUL)

                # ---- W_o for this chunk: any leftovers from the previous
                # chunk, then defer this chunk's 4 s-blocks into the next
                # chunk's attention (keeps the PE fed through ACT stalls) ----
                for sb_i in wo_pending:
                    emit_wo(sb_i, act_side=True)
                wo_pending = [sc * 4 + sbl for sbl in range(4)]

            for sb_i in wo_pending:  # last chunk's W_o at the tail
                emit_wo(sb_i, act_side=True)

    _legalize_waits(nc)
    return nc


def _legalize_waits(nc, max_waits=1):
    """Split >max_waits sync waits onto preceding same-engine NoOps
    (several instruction encodings only have one sync-wait slot)."""
    for fn in nc.m.functions:
        for bb in fn.blocks:
            new_insts = []
            for inst in bb.instructions:
                si = inst.sync_info
                waits = list(si.on_wait) if si is not None and si.on_wait else []
                if len(waits) > max_waits:
                    carry, keep = waits[:-max_waits], waits[-max_waits:]
                    for i, w in enumerate(carry):
                        new_insts.append(mybir.InstNoOp(
                            name=f"{inst.name}_wsplit{i}",
                            engine=inst.engine,
                            bass_nofuse=True,
                            sync_info=mybir.SyncInfo(on_wait=[w], on_update=[]),
                        ))
                    si.on_wait = keep
                new_insts.append(inst)
            bb.instructions[:] = new_insts


def _host_constants():
    # RoPE channel permutation: row r (within a head, 0..63) holds source
    # channel d = 2*i + odd with i = 16*(r//32) + r%16, odd = (r%32)//16.
    r = np.arange(64)
    i_ = 16 * (r // 32) + (r % 16)
    odd = (r % 32) // 16
    dsrc = 2 * i_ + odd  # source channel per permuted row

    inv_freq = ROPE_BASE ** (-(i_.astype(np.float64)) * 2.0 / Dh)
    ang = np.arange(S, dtype=np.float64)[None, :] * inv_freq[:, None]  # [64, S]
    cos64 = np.cos(ang)
    sin64 = np.sin(ang) * np.where(odd == 0, -1.0, 1.0)[:, None]
    cosd = np.tile(cos64, (2, 1)).astype(DT_NP)
    sins = np.tile(sin64, (2, 1)).astype(DT_NP)

    t = np.arange(128)[:, None]
    s = np.arange(128)[None, :]
    tri = (t <= s).astype(DT_NP)

    sel2 = np.zeros((33, 128), DT_NP)
    sel2[0, 0:64] = 1
    sel2[32, 64:128] = 1
    return dsrc, cosd, sins, tri, sel2


_CACHE = {}


def _run(inputs, trace=False):
    if "nc" not in _CACHE:
        _CACHE["nc"] = _build_program()
        _CACHE["consts"] = _host_constants()
    nc = _CACHE["nc"]
    dsrc, cosd, sins, tri, sel2 = _CACHE["consts"]

    x = np.ascontiguousarray(np.asarray(inputs["x"]), dtype=np.float32)
    W_q = np.asarray(inputs["W_q"], dtype=np.float32)
    W_k = np.asarray(inputs["W_k"], dtype=np.float32)
    W_v = np.asarray(inputs["W_v"], dtype=np.float32)
    W_o = np.asarray(inputs["W_o"], dtype=np.float32)

    # x^T per batch in device layout [p, chunk, eo, s] (contiguous DMA lines)
    xT = []
    for b in range(B):
        xb = np.ascontiguousarray(x[b].T).astype(DT_NP)       # [E, S]
        xT.append(np.ascontiguousarray(
            xb.reshape(8, 128, NCHUNK, 512).transpose(1, 2, 0, 3)))

    def wlayout(Wrows):  # [256, E] -> device [128, 8, 256]
        wt = np.ascontiguousarray(Wrows.T).astype(DT_NP)      # [E, 256]
        return np.ascontiguousarray(wt.reshape(8, 128, HD).transpose(1, 0, 2))

    in_maps = []
    for c in range(8):
        b, g = divmod(c, 4)
        heads = np.arange(4 * g, 4 * g + 4)
        rows_qk = (heads[:, None] * 64 + dsrc[None, :]).reshape(-1)   # permuted
        rows_v = (heads[:, None] * 64 + np.arange(64)[None, :]).reshape(-1)
        wot = np.ascontiguousarray(W_o[:, rows_v].T).astype(DT_NP)    # [256, E]
        in_maps.append({
            "xT": xT[b],
            "wq": wlayout(W_q[rows_qk]),
            "wk": wlayout(W_k[rows_qk]),
            "wv": wlayout(W_v[rows_v]),
            "wo": np.ascontiguousarray(wot.reshape(2, 128, E).transpose(1, 0, 2)),
            "cosd": cosd, "sins": sins, "tric": tri, "sel2c": sel2,
        })

    res = bass_utils.run_bass_kernel_spmd(
        nc, in_maps, core_ids=list(range(8)), trace=trace,
    )
    out = np.zeros((B, S, E), np.float32)
    for c in range(8):
        out[c // 4] += res.results[c]["y"].astype(np.float32)
    return out, res


def kernel(**inputs):
    out, _ = _run(inputs, trace=False)
    return out


# revision 39
# speedup vs baseline: 1.3037x; 1.0193x over previous
"""Multi-head causal attention with RoPE on 8 TRN2 NeuronCores.

Sharding: batch (2) x head-groups (4 of 4 heads) -> 8 cores.
Per core, processed per 512-row s-chunk with everything interleaved to keep
the PE array dense: QKV projection for the chunk, RoPE (stream_shuffle +
sign-folded cos/sin, all bf16 on DVE), row-tiled scores S^T = Kr @ Qr^T --
the two heads of a pair run CONCURRENTLY as K=64 matmuls in PE row-groups
0-1 / 2-3 -- with causal block-skip and diagonal-range narrowing, one wide
fused exp over both heads' scores from a 2-bank PSUM tile, a single
128x128 triangle mask applied in-place post-exp, PV matmul with a
ones-column on V accumulating the softmax denominator, DVE reciprocal,
ones-matmul broadcast, then the W_o partial projection for the chunk.
Host sums the 4 per-batch partials.
"""
import os
import sys

sys.path.insert(0, "/opt/trn_rl_repo")

import ml_dtypes
import numpy as np

import concourse.bass as bass
import concourse.mybir as mybir
import concourse.tile as tile
from concourse import bass_utils

F32 = mybir.dt.float32
F32R = mybir.dt.float32r
BF16 = mybir.dt.bfloat16

DT_NAME = os.environ.get("ATTN_DT", "bf16")
DT = {"f32r": F32R, "bf16": BF16}[DT_NAME]
DT_NP = {"f32r": np.float32, "bf16": ml_dtypes.bfloat16}[DT_NAME]

B, S, E, H, Dh = 2, 2048, 1024, 16, 64
HG = 4            # heads per core
HD = HG * Dh      # 256 output channels per core
SCALE = float(1.0 / np.sqrt(np.float32(1024.0)))
ROPE_BASE = 10000.0
NCHUNK = S // 512     # 4 s-chunks of 512
NTB = S // 128        # 16 t-blocks of 128
SHUF16 = list(range(16, 32)) + list(range(0, 16))

Exp = mybir.ActivationFunctionType.Exp
Ln = mybir.ActivationFunctionType.Ln
MUL = mybir.AluOpType.mult
ADD = mybir.AluOpType.add


def _build_program():
    nc = bass.Bass("TRN2", target_bir_lowering=False, debug=False)

    xT = nc.dram_tensor("xT", [128, NCHUNK, 8, 512], DT, kind="ExternalInput")
    wq = nc.dram_tensor("wq", [128, 8, HD], DT, kind="ExternalInput")
    wk = nc.dram_tensor("wk", [128, 8, HD], DT, kind="ExternalInput")
    wv = nc.dram_tensor("wv", [128, 8, HD], DT, kind="ExternalInput")
    wo = nc.dram_tensor("wo", [128, 2, E], DT, kind="ExternalInput")
    cosd = nc.dram_tensor("cosd", [128, S], DT, kind="ExternalInput")
    sins = nc.dram_tensor("sins", [128, S], DT, kind="ExternalInput")
    tric = nc.dram_tensor("tric", [128, 128], DT, kind="ExternalInput")
    sel2c = nc.dram_tensor("sel2c", [33, 128], DT, kind="ExternalInput")
    y = nc.dram_tensor("y", [S, E], DT, kind="ExternalOutput")

    with tile.TileContext(nc) as tc:
        with (
            tc.tile_pool(name="persist", bufs=1) as pp,
            tc.tile_pool(name="xchunks", bufs=2) as xp,
            tc.tile_pool(name="ropetmp", bufs=3) as rt,
            tc.tile_pool(name="att_es", bufs=3) as ep,
            tc.tile_pool(name="att_row", bufs=2) as rp,
            tc.tile_pool(name="ystg", bufs=2) as yp,
            tc.tile_pool(name="ps_proj", bufs=1, space="PSUM") as ps1,
            tc.tile_pool(name="ps_sc", bufs=2, space="PSUM") as ps_s,
            tc.tile_pool(name="ps_ot", bufs=1, space="PSUM") as ps_o,
            tc.tile_pool(name="ps_aux", bufs=1, space="PSUM") as ps_a,
        ):
            # ---- persistent tensors ----
            qz = pp.tile([128, 2, S], DT)    # Qr^T per head-pair
            krt = pp.tile([128, 2, S], DT)   # Kr^T
            vau = pp.tile([128, NTB, HG, 65], DT)  # V + ones col per (tb, h)
            ot = pp.tile([128, 2, S], DT)    # O^T normalized

            # critical-path DMAs on the scalar queue: Q/K weights first so
            # the first projection can start as soon as chunk 0 of x lands
            # (x chunks + y output stream on the sync queue).
            wq_sb = pp.tile([128, 8, HD], DT)
            nc.scalar.dma_start(wq_sb[:], wq.ap())
            wk_sb = pp.tile([128, 8, HD], DT)
            nc.scalar.dma_start(wk_sb[:], wk.ap())
            cos_sb = pp.tile([128, S], DT)
            nc.scalar.dma_start(cos_sb[:], cosd.ap())
            sin_sb = pp.tile([128, S], DT)
            nc.scalar.dma_start(sin_sb[:], sins.ap())
            # V/O weights + cold-path constants on the gpsimd software DGE
            wv_sb = pp.tile([128, 8, HD], DT)
            nc.gpsimd.dma_start(wv_sb[:], wv.ap())
            wo_sb = pp.tile([128, 2, E], DT)
            nc.gpsimd.dma_start(wo_sb[:], wo.ap())
            sel2_sb = pp.tile([33, 128], DT)
            nc.gpsimd.dma_start(sel2_sb[:], sel2c.ap())
            mask_sb = pp.tile([128, 128], DT)
            nc.gpsimd.dma_start(mask_sb[:], tric.ap())

            # reciprocal rows live at partitions 0 and 32 (DVE/ACT partition
            # shifts must be 32-granular; the denominators sit at p=64)
            rows2 = pp.tile([33, 512], DT)  # bf16 recip rows for bc matmul
            with nc.allow_low_precision(reason="rounded matmul input"):
                nc.vector.memset(rows2[:], 0.0)
            heat_sb = pp.tile([128, 128], DT)
            nc.vector.memset(heat_sb[:], 0.0)

            def heat(target, n=10):
                # full-array 128x128 matmuls to trip the HAM activity window
                # back to K=8/8. Scratch lands in `target` PSUM, whose next
                # real matmul uses start=True and overwrites it.
                for _ in range(n):
                    nc.tensor.matmul(target[:, 0:128], heat_sb[:],
                                     heat_sb[:], start=True, stop=True)

            # warm the PE during the initial DMA streams; sized to bridge
            # until the first x chunk + W_q land (~13us)
            hstart = ps_s.tile([128, 1024], F32, tag="pss", name="heatstart")
            heat(hstart, n=58)

            # ones column of V_aug (free-dim broadcast from a [128,1] slice)
            ones_sb = pp.tile([128, 1], DT)
            with nc.allow_low_precision(reason="rounded matmul input"):
                nc.vector.memset(ones_sb[:], 1.0)
                nc.vector.tensor_copy(
                    vau[:, :, :, 64:65],
                    ones_sb[:, 0:1].to_broadcast((128, NTB, HG, 1)),
                )

            def emit_wo(sb_i, act_side):
                # W_o partial projection for one 128-row s-block
                tsl = slice(sb_i * 128, (sb_i + 1) * 128)
                ystg = yp.tile([128, E], DT, tag="y")
                for ec in range(2):
                    pool, tg = (ps_a, "aux") if ec == 0 else (ps1, "pq")
                    py = pool.tile([128, 512], F32, tag=tg, name="py")
                    for blk in range(2):
                        nc.tensor.matmul(
                            py[:], ot[:, blk, tsl],
                            wo_sb[:, blk, ec * 512:(ec + 1) * 512],
                            start=(blk == 0), stop=(blk == 1),
                        )
                    with nc.allow_low_precision(reason="rounded matmul input"):
                        if ec == 1 and act_side:
                            nc.scalar.copy(ystg[:, 512:1024], py[:])
                        else:
                            nc.vector.tensor_copy(
                                ystg[:, ec * 512:(ec + 1) * 512], py[:])
                nc.sync.dma_start(y.ap()[tsl, :], ystg[:])

            wo_pending = []  # W_o s-blocks deferred into the next chunk

            for sc in range(NCHUNK):
                ss = slice(sc * 512, (sc + 1) * 512)
                ntb = 4 * sc + 4

                # ---- projection for this chunk (x alone on the sync queue) ----
                xc = xp.tile([128, 8, 512], DT, tag="xc")
                nc.sync.dma_start(xc[:], xT.ap()[:, sc])

                # Q and K projections with RoPE (all-bf16 DVE chain -> 2x)
                for w_sb, dst in ((wq_sb, qz), (wk_sb, krt)):
                    for mb in range(2):
                        pq = ps1.tile([128, 512], F32, tag="pq")
                        for e in range(8):
                            nc.tensor.matmul(
                                pq[:], w_sb[:, e, mb * 128:(mb + 1) * 128],
                                xc[:, e, :], start=(e == 0), stop=(e == 7),
                            )
                        a = rt.tile([128, 512], DT, tag="a")
                        with nc.allow_low_precision(reason="rounded matmul input"):
                            nc.vector.tensor_copy(a[:], pq[:])
                        bsh = rt.tile([128, 512], DT, tag="b")
                        nc.vector.stream_shuffle(bsh[:], a[:], SHUF16)
                        t1 = rt.tile([128, 512], DT, tag="t1")
                        t2 = rt.tile([128, 512], DT, tag="t2")
                        with nc.allow_low_precision(reason="rounded matmul input"):
                            nc.vector.tensor_tensor(t1[:], bsh[:], sin_sb[:, ss], MUL)
                            nc.vector.tensor_tensor(t2[:], a[:], cos_sb[:, ss], MUL)
                            nc.vector.tensor_tensor(dst[:, mb, ss], t2[:], t1[:], ADD)

                # V projection (re-uses the pq bank)
                for tbl in range(4):
                    tb = sc * 4 + tbl
                    pvt = ps1.tile([128, 512], F32, tag="pq", name="pv")
                    pv = pvt[:, 0:256]
                    for e in range(8):
                        nc.tensor.matmul(
                            pv, xc[:, e, tbl * 128:(tbl + 1) * 128],
                            wv_sb[:, e, :], start=(e == 0), stop=(e == 7),
                        )
                    with nc.allow_low_precision(reason="rounded matmul input"):
                        nc.scalar.copy(
                            vau[:, tb, :, 0:64],
                            pv.rearrange("p (h d) -> p h d", d=64),
                        )

                # ---- attention for this chunk, one head-pair at a time ----
                # the two heads of a pair run concurrently as K=64 matmuls in
                # PE row-groups 0-1 (partitions 0:64) and 2-3 (64:128), into
                # the two banks of one [128,1024] PSUM tile.
                wo_at = ()  # all W_o filler goes to the hp-end stall
                for hp in range(2):
                    otp = [ps_o.tile([65, 512], F32, tag=f"ot{hi}", name=f"otp{hi}")
                           for hi in range(2)]
                    for tb in range(ntb):
                        m = tb - 4 * sc
                        if tb in wo_at and wo_pending:
                            emit_wo(wo_pending.pop(0), act_side=False)
                        lo = max(m, 0) * 128  # first valid column (diag blocks)
                        tsl = slice(tb * 128, (tb + 1) * 128)
                        scol = slice(sc * 512 + lo, (sc + 1) * 512)
                        pss = ps_s.tile([128, 1024], F32, tag="pss", name="pss")
                        for hi in range(2):
                            hsl = slice(64 * hi, 64 * hi + 64)
                            nc.tensor.matmul(
                                pss[:, 512 * hi + lo:512 * (hi + 1)],
                                krt[hsl, hp, tsl], qz[hsl, hp, scol],
                                start=True, stop=True,
                            )
                        es = ep.tile([128, 1024], DT, tag="es", name="es")
                        with nc.allow_low_precision(reason="rounded matmul input"):
                            if lo:
                                pr = pss[:].rearrange("p (h s) -> p h s", h=2)
                                er = es[:].rearrange("p (h s) -> p h s", h=2)
                                nc.scalar.activation(er[:, :, lo:512], pr[:, :, lo:512],
                                                     Exp, bias=0.0, scale=SCALE)
                            else:
                                nc.scalar.activation(es[:], pss[:],
                                                     Exp, bias=0.0, scale=SCALE)
                        if m >= 0:  # diagonal block: mask the 128-wide triangles
                            with nc.allow_low_precision(reason="rounded matmul input"):
                                for hi in range(2):
                                    dsl = slice(512 * hi + lo, 512 * hi + lo + 128)
                                    nc.vector.tensor_tensor(
                                        es[:, dsl], es[:, dsl], mask_sb[:], MUL)
                        for hi in range(2):
                            nc.tensor.matmul(
                                otp[hi][:, lo:512], vau[:, tb, 2 * hp + hi, :],
                                es[:, 512 * hi + lo:512 * (hi + 1)],
                                start=(tb == 0), stop=(tb == ntb - 1),
                            )
                    # W_o filler covers the ln/exp latency before bc
                    for _ in range(2):
                        if wo_pending:
                            emit_wo(wo_pending.pop(0), act_side=False)
                    # 1/colsum via ACT exp(-ln); one selector matmul then
                    # broadcasts both rows to the full 128-block.
                    for hi in range(2):
                        lnr = rp.tile([1, 512], F32, tag="lnr", name=f"lnr{hi}")
                        nc.scalar.activation(lnr[:], otp[hi][64:65, :],
                                             Ln, bias=0.0, scale=1.0)
                        with nc.allow_low_precision(reason="rounded matmul input"):
                            nc.scalar.activation(rows2[32 * hi:32 * hi + 1, :],
                                                 lnr[:], Exp, bias=0.0, scale=-1.0)
                    bc = ps_a.tile([128, 512], F32, tag="aux", name="bc")
                    nc.tensor.matmul(bc[:], sel2_sb[:], rows2[:], start=True, stop=True)
                    # normalize both heads: O rows are otp[hi][0:64]
                    for hi in range(2):
                        bcs = rp.tile([64, 512], F32, tag="bcs", name=f"bcs{hi}")
                        nc.vector.tensor_copy(bcs[:], bc[hi * 64:(hi + 1) * 64, :])
                        with nc.allow_low_precision(reason="rounded matmul input"):
                            nc.vector.tensor_tensor(ot[hi * 64:(hi + 1) * 64, hp, ss],
                                                    otp[hi][0:64, :], bcs[:], MUL)

                # ---- W_o for this chunk: any leftovers from the previous
                # chunk, then defer this chunk's 4 s-blocks into the next
                # chunk's attention (keeps the PE fed through ACT stalls) ----
                for sb_i in wo_pending:
                    emit_wo(sb_i, act_side=True)
                wo_pending = [sc * 4 + sbl for sbl in range(4)]

            for sb_i in wo_pending:  # last chunk's W_o at the tail
                emit_wo(sb_i, act_side=True)

    _legalize_waits(nc)
    return nc


def _legalize_waits(nc, max_waits=1):
    """Split >max_waits sync waits onto preceding same-engine NoOps
    (several instruction encodings only have one sync-wait slot)."""
    for fn in nc.m.functions:
        for bb in fn.blocks:
            new_insts = []
            for inst in bb.instructions:
                si = inst.sync_info
                waits = list(si.on_wait) if si is not None and si.on_wait else []
                if len(waits) > max_waits:
                    carry, keep = waits[:-max_waits], waits[-max_waits:]
                    for i, w in enumerate(carry):
                        new_insts.append(mybir.InstNoOp(
                            name=f"{inst.name}_wsplit{i}",
                            engine=inst.engine,
                            bass_nofuse=True,
                            sync_info=mybir.SyncInfo(on_wait=[w], on_update=[]),
                        ))
                    si.on_wait = keep
                new_insts.append(inst)
            bb.instructions[:] = new_insts


def _host_constants():
    # RoPE channel permutation: row r (within a head, 0..63) holds source
    # channel d = 2*i + odd with i = 16*(r//32) + r%16, odd = (r%32)//16.
    r = np.arange(64)
    i_ = 16 * (r // 32) + (r % 16)
    odd = (r % 32) // 16
    dsrc = 2 * i_ + odd  # source channel per permuted row

    inv_freq = ROPE_BASE ** (-(i_.astype(np.float64)) * 2.0 / Dh)
    ang = np.arange(S, dtype=np.float64)[None, :] * inv_freq[:, None]  # [64, S]
    cos64 = np.cos(ang)
    sin64 = np.sin(ang) * np.where(odd == 0, -1.0, 1.0)[:, None]
    cosd = np.tile(cos64, (2, 1)).astype(DT_NP)
    sins = np.tile(sin64, (2, 1)).astype(DT_NP)

    t = np.arange(128)[:, None]
    s = np.arange(128)[None, :]
    tri = (t <= s).astype(DT_NP)

    sel2 = np.zeros((33, 128), DT_NP)
    sel2[0, 0:64] = 1
    sel2[32, 64:128] = 1
    return dsrc, cosd, sins, tri, sel2


_CACHE = {}


def _run(inputs, trace=False):
    if "nc" not in _CACHE:
        _CACHE["nc"] = _build_program()
        _CACHE["consts"] = _host_constants()
    nc = _CACHE["nc"]
    dsrc, cosd, sins, tri, sel2 = _CACHE["consts"]

    x = np.ascontiguousarray(np.asarray(inputs["x"]), dtype=np.float32)
    W_q = np.asarray(inputs["W_q"], dtype=np.float32)
    W_k = np.asarray(inputs["W_k"], dtype=np.float32)
    W_v = np.asarray(inputs["W_v"], dtype=np.float32)
    W_o = np.asarray(inputs["W_o"], dtype=np.float32)

    # x^T per batch in device layout [p, chunk, eo, s] (contiguous DMA lines)
    xT = []
    for b in range(B):
        xb = np.ascontiguousarray(x[b].T).astype(DT_NP)       # [E, S]
        xT.append(np.ascontiguousarray(
            xb.reshape(8, 128, NCHUNK, 512).transpose(1, 2, 0, 3)))

    def wlayout(Wrows):  # [256, E] -> device [128, 8, 256]
        wt = np.ascontiguousarray(Wrows.T).astype(DT_NP)      # [E, 256]
        return np.ascontiguousarray(wt.reshape(8, 128, HD).transpose(1, 0, 2))

    in_maps = []
    for c in range(8):
        b, g = divmod(c, 4)
        heads = np.arange(4 * g, 4 * g + 4)
        rows_qk = (heads[:, None] * 64 + dsrc[None, :]).reshape(-1)   # permuted
        rows_v = (heads[:, None] * 64 + np.arange(64)[None, :]).reshape(-1)
        wot = np.ascontiguousarray(W_o[:, rows_v].T).astype(DT_NP)    # [256, E]
        in_maps.append({
            "xT": xT[b],
            "wq": wlayout(W_q[rows_qk]),
            "wk": wlayout(W_k[rows_qk]),
            "wv": wlayout(W_v[rows_v]),
            "wo": np.ascontiguousarray(wot.reshape(2, 128, E).transpose(1, 0, 2)),
            "cosd": cosd, "sins": sins, "tric": tri, "sel2c": sel2,
        })

    res = bass_utils.run_bass_kernel_spmd(
        nc, in_maps, core_ids=list(range(8)), trace=trace,
    )
    out = np.zeros((B, S, E), np.float32)
    for c in range(8):
        out[c // 4] += res.results[c]["y"].astype(np.float32)
    return out, res


def kernel(**inputs):
    out, _ = _run(inputs, trace=False)
    return out
